# revision 1
# baseline (speedup 1.0000x reference)
"""Trainium2 Bass kernel for nn_DecoupledTextDecoder.

Reference computation (per batch sample b, nB=256, nC=512, nH*nW=512, nT=40,
nCls=97):
  A_n   = A / sum_hw(A)                       (attention normalize)
  C     = einsum('chw,thw->tc', feature_b, A_n_b)       [40, 512]
  hidden= C @ W.T + b                                   [40, 512]
  cfP   = hidden @ protos.T                             [40, 97]
  cfCos = cfP / (||hidden||_row + EPS)
  outCls= concat([cfP * ALPHA, UNK], -1); outCos = concat([cfCos, UNK], -1)
  ragged-pack the first textLength[b] rows of each sample into one buffer.

Strategy: data-parallel over nB across 8 NeuronCores (32 samples/core).
The hw-contraction needs hw on SBUF partitions for the PE, so feature and A
are uploaded pre-transposed ([b, hw, c] / [b, hw, t]) — a host-side layout
choice — removing all on-chip transposes.  The attention normalization is
algebraically folded into a per-column scale s[t]=1/rowsum(A) applied to C
(host computes s exactly in fp32).  Per-sample matmul chain on the PE:
  M1: C^T[c,t]      = FT-chunks(lhsT) x AT-chunks(rhs), accumulated over hw
  M2: hidden^T[c',t]= WT-chunks(lhsT) x C^T(rhs),       accumulated over c
  M3: cfP[t,cls]    = hidden^T-slices(lhsT) x protos^T(rhs), accum over c'
  hnorm^2 via DVE squares + ones-matmul partition reduce -> [t,1] layout,
  so the cfCos division and ALPHA scale are per-partition tensor_scalars.
The ragged pack is pure data movement with runtime row offsets; it is done
on the host with a vectorized scatter (the per-sample rows shard cleanly).

Matmul operands use fp16 (inputs rounded on host / on copy); accumulation
is fp32 in PSUM and everything after M3 stays fp32.  Measured end-to-end
resid-var vs the fp32 reference is ~1e-7.
"""

import numpy as np

import concourse.bass as bass
import concourse.bacc as bacc
import concourse.tile as tile
import concourse.mybir as mybir
from concourse.bass_utils import run_bass_kernel_spmd

F32 = mybir.dt.float32
EPS = 0.0009

N_CORES = 8
NB = 256
NB_C = NB // N_CORES       # samples per core
NC = 512                   # channels
HW = 512                   # nH*nW
NT = 40                    # text steps
NCLS = 97
D = NCLS + 1

GROUP = 2                  # samples per feature DMA (1 MiB transfers)
BLOCKS = [12, 12, 8]       # samples per block (sum = NB_C)
assert sum(BLOCKS) == NB_C and all(b % GROUP == 0 for b in BLOCKS)


def _mgroups(ns):
    """Partition-dim groups of t-columns for M3 (<=128 each, 40-aligned)."""
    w = ns * NT
    out, o = [], 0
    while o < w:
        m = min(120, w - o)
        out.append((o, m))
        o += m
    return out


def build_kernel(dt2=mybir.dt.float16, reps=1, group=GROUP, dual_ring=False,
                 ft_bufs=3, timing_mode=False, hw_loop=0, rings=None,
                 out_rings=None, skip_load=False, skip_compute=False,
                 dup_dma=False, dup_m1=False, at_g=False, flat_ft=False,
                 at_hoist=False, split2=False, dt_ft=None):
    """Build + compile the per-core Bass program. Returns nc.

    timing_mode=True replaces the bulk inputs with Internal DRAM scratch so
    repeated-execution benchmarks don't pay host->device re-transfers; the
    on-device HBM traffic is identical.
    """
    nc = bacc.Bacc("TRN2", target_bir_lowering=False, debug=False,
                   enable_asserts=True, num_devices=N_CORES)

    if dt_ft is None:
        dt_ft = dt2
    kind_b = "Internal" if timing_mode else "ExternalInput"
    ft = nc.dram_tensor("ft", [NB_C * HW, NC], dt_ft, kind=kind_b).ap()
    at = nc.dram_tensor("at", [NB_C * HW, NT], dt_ft, kind=kind_b).ap()
    wt = nc.dram_tensor("wt", [NC, NC], dt2, kind=kind_b).ap()
    pt = nc.dram_tensor("pt", [NC, NCLS], dt2, kind=kind_b).ap()
    bcol = nc.dram_tensor("bcol", [128, NC // 128], F32, kind=kind_b).ap()
    srow = nc.dram_tensor("srow", [1, NB_C * NT], F32, kind=kind_b).ap()
    au = nc.dram_tensor("au", [1, 2], F32, kind="ExternalInput").ap()
    ocls = nc.dram_tensor("ocls", [NB_C * NT, D], F32, kind="ExternalOutput").ap()
    ocos = nc.dram_tensor("ocos", [NB_C * NT, D], F32, kind="ExternalOutput").ap()

    with tile.TileContext(nc) as tc:
        with (
            tc.tile_pool(name="consts", bufs=1) as consts,
            tc.tile_pool(name="ftp", bufs=ft_bufs) as ftp,
            tc.tile_pool(name="atp", bufs=3) as atp,
            tc.tile_pool(name="work", bufs=8) as work,
            tc.tile_pool(name="sqp", bufs=5) as sqp,
            tc.tile_pool(name="outp", bufs=12) as outp,
            tc.tile_pool(name="ps_ct", bufs=4, space="PSUM") as ps_ct,
            tc.tile_pool(name="ps_h", bufs=2, space="PSUM") as ps_h,
            tc.tile_pool(name="ps_p", bufs=1, space="PSUM") as ps_p,
            tc.tile_pool(name="ps_sq", bufs=1, space="PSUM") as ps_sq,
        ):
            if rings is None:
                ring_eng = [nc.scalar, nc.sync] if dual_ring else [nc.sync]
            else:
                emap = {"s": nc.sync, "a": nc.scalar, "g": nc.gpsimd}
                ring_eng = [emap[ch] for ch in rings]
            if out_rings is None:
                oring_eng = [nc.sync]
            else:
                emap = {"s": nc.sync, "a": nc.scalar, "g": nc.gpsimd}
                oring_eng = [emap[ch] for ch in out_rings]

            def emit():
                _emit_once(nc, tc, consts, ftp, atp, work, sqp, outp,
                           ps_ct, ps_h, ps_p, ps_sq,
                           ft, at, wt, pt, bcol, srow, au, ocls, ocos, dt2,
                           group, ring_eng, oring_eng, skip_load, skip_compute,
                           dup_dma, dup_m1, at_g, flat_ft, at_hoist, split2,
                           dt_ft)

            if hw_loop:
                with tc.For_i(0, hw_loop, 1):
                    emit()
            else:
                for _ in range(reps):
                    emit()
    nc.compile()
    return nc


def _emit_once(nc, tc, consts, ftp, atp, work, sqp, outp,
               ps_ct, ps_h, ps_p, ps_sq,
               ft, at, wt, pt, bcol, srow, au, ocls, ocos, dt2,
               group, ring_eng, oring_eng, skip_load=False, skip_compute=False,
               dup_dma=False, dup_m1=False, at_g=False, flat_ft=False,
               at_hoist=False, split2=False, dt_ft=None):
    if dt_ft is None:
        dt_ft = dt2
    mult = mybir.AluOpType.mult
    add = mybir.AluOpType.add

    # ---- constants -------------------------------------------------------
    ones_row = consts.tile([1, 128], F32, tag="ones_row")
    nc.vector.memset(ones_row[:], 1.0)
    ones_col = consts.tile([128, 1], F32, tag="ones_col")
    nc.vector.memset(ones_col[:], 1.0)

    wt_sb = []
    for k in range(4):
        t = consts.tile([128, NC], dt2, tag=f"wt{k}")
        nc.sync.dma_start(out=t[:], in_=wt[k * 128:(k + 1) * 128, :])
        wt_sb.append(t)
    pt_sb = []
    for k in range(4):
        t = consts.tile([128, NCLS], dt2, tag=f"pt{k}")
        nc.sync.dma_start(out=t[:], in_=pt[k * 128:(k + 1) * 128, :])
        pt_sb.append(t)
    b_sb = consts.tile([128, 4], F32, tag="b")
    nc.sync.dma_start(out=b_sb[:], in_=bcol[:])
    s_sb = consts.tile([1, NB_C * NT], F32, tag="s")
    nc.sync.dma_start(out=s_sb[:], in_=srow[:])
    au_sb = consts.tile([1, 2], F32, tag="au")
    nc.sync.dma_start(out=au_sb[:], in_=au[:])

    # Broadcast s over partitions via k=1 matmul: S_all[p, col] = s[col].
    s_all = consts.tile([128, NB_C * NT], F32, tag="s_all")
    o = 0
    while o < NB_C * NT:
        w = min(512, NB_C * NT - o)
        ps = ps_h.tile([128, 512], F32, tag="h")
        nc.tensor.matmul(ps[:, :w], ones_row[:], s_sb[:, o:o + w],
                         start=True, stop=True)
        nc.vector.tensor_copy(s_all[:, o:o + w], ps[:, :w])
        o += w
    # alpha / unk broadcast columns
    au_ps = ps_h.tile([128, 512], F32, tag="h")
    nc.tensor.matmul(au_ps[:, :2], ones_row[:], au_sb[:], start=True, stop=True)
    au_col = consts.tile([128, 2], F32, tag="au_col")
    nc.vector.tensor_copy(au_col[:], au_ps[:, :2])
    alpha_col = au_col[:, 0:1]
    unk_col = au_col[:, 1:2]

    at_all = None
    if at_hoist:
        at_all = atp.tile([128, NB_C * 4, NT], dt_ft, tag="at_all")
        half = NB_C * HW // 2
        ring_eng[0].dma_start(
            out=at_all[:, :NB_C * 2, :],
            in_=at[0:half, :].rearrange("(g p) t -> p g t", p=128))
        ring_eng[-1].dma_start(
            out=at_all[:, NB_C * 2:, :],
            in_=at[half:NB_C * HW, :].rearrange("(g p) t -> p g t", p=128))

    # ---- main loop over sample blocks -----------------------------------
    s0 = 0
    for ns in BLOCKS:
        w = ns * NT
        col0 = s0 * NT

        # feature/attention loads, `group` samples per DMA
        ftg, atg = [], []
        for j in range(ns // group):
            r0 = (s0 + j * group) * HW
            eng = ring_eng[j % len(ring_eng)]
            ftile = ftp.tile([128, group * 4, NC], dt_ft, tag="ft")
            atile = atp.tile([128, group * 4, NT], dt_ft, tag="at")
            if not skip_load:
                if split2:
                    hwrows = group * HW // 2
                    ring_eng[0].dma_start(
                        out=ftile[:, :group * 2, :],
                        in_=ft[r0:r0 + hwrows, :].rearrange("(g p) c -> p g c", p=128))
                    ring_eng[-1].dma_start(
                        out=ftile[:, group * 2:, :],
                        in_=ft[r0 + hwrows:r0 + group * HW, :].rearrange("(g p) c -> p g c", p=128))
                elif flat_ft:
                    for q in range(group * 4):
                        ring_eng[(j * group * 4 + q) % len(ring_eng)].dma_start(
                            out=ftile[:, q, :],
                            in_=ft[r0 + q * 128:r0 + (q + 1) * 128, :])
                else:
                    eng.dma_start(
                        out=ftile[:],
                        in_=ft[r0:r0 + group * HW, :].rearrange("(g p) c -> p g c", p=128))
                if not at_hoist:
                    (nc.gpsimd if at_g else eng).dma_start(
                        out=atile[:],
                        in_=at[r0:r0 + group * HW, :].rearrange("(g p) t -> p g t", p=128))
                if dup_dma:
                    dtile = ftp.tile([128, group * 4, NC], dt_ft, tag="ftdup", name="dtile")
                    eng.dma_start(
                        out=dtile[:],
                        in_=ft[r0:r0 + group * HW, :].rearrange("(g p) c -> p g c", p=128))
            ftg.append(ftile)
            atg.append(atile)
        if skip_compute:
            s0 += ns
            continue

        # M1: C_raw^T accumulated into 4 psum banks, one 40-col slice/sample
        ct_ps = [ps_ct.tile([128, 480], F32, tag="ct", name=f"ct_ps{jj}") for jj in range(4)]
        for sl in range(ns):
            ftile = ftg[sl // group]
            h = sl % group
            if at_hoist:
                atile, abase = at_all, (s0 + sl) * 4
            else:
                atile, abase = atg[sl // group], (sl % group) * 4
            for rep2 in range(2 if dup_m1 else 1):
                for jj in range(4):
                    for kk in range(4):
                        nc.tensor.matmul(
                            ct_ps[jj][:, sl * NT:(sl + 1) * NT],
                            ftile[:, h * 4 + kk, jj * 128:(jj + 1) * 128],
                            atile[:, abase + kk, :],
                            start=(kk == 0), stop=(kk == 3))

        # scale by s (normalization fold) + cast to dt2
        ct_sb = []
        for jj in range(4):
            t = work.tile([128, 480], dt2, tag="ctsb")
            nc.vector.tensor_tensor(t[:, :w], ct_ps[jj][:, :w],
                                    s_all[:, col0:col0 + w], mult)
            ct_sb.append(t)

        # M2: hidden^T (no bias yet), 4 psum banks
        h_sb = []
        sq = []
        for jj in range(4):
            hp = ps_h.tile([128, 480], F32, tag="h")
            for kk in range(4):
                nc.tensor.matmul(hp[:, :w], wt_sb[kk][:, jj * 128:(jj + 1) * 128],
                                 ct_sb[kk][:, :w], start=(kk == 0), stop=(kk == 3))
            # bias add on ACT during psum->sbuf copy (cast to dt2)
            hs = work.tile([128, 480], dt2, tag="hsb")
            nc.scalar.activation(hs[:, :w], hp[:, :w],
                                 mybir.ActivationFunctionType.Identity,
                                 bias=b_sb[:, jj:jj + 1])
            h_sb.append(hs)
            # squared hidden for the row norms
            st = sqp.tile([128, 480], F32, tag="sq")
            nc.vector.tensor_tensor(st[:, :w], hs[:, :w], hs[:, :w], mult)
            sq.append(st)
        nc.vector.tensor_tensor(sq[0][:, :w], sq[0][:, :w], sq[1][:, :w], add)
        nc.vector.tensor_tensor(sq[2][:, :w], sq[2][:, :w], sq[3][:, :w], add)
        nc.vector.tensor_tensor(sq[0][:, :w], sq[0][:, :w], sq[2][:, :w], add)

        mg = _mgroups(ns)
        # partition-reduce -> hnorm^2 in [t, 1] layout
        sq_ps = ps_sq.tile([128, len(mg)], F32, tag="sqc")
        for g, (o, m) in enumerate(mg):
            nc.tensor.matmul(sq_ps[:m, g:g + 1], sq[0][:, o:o + m], ones_col[:],
                             start=True, stop=True)
        # r = 1 / (sqrt(hnorm^2) + EPS)
        rcols = work.tile([128, len(mg)], F32, tag="rc")
        for g, (o, m) in enumerate(mg):
            nc.scalar.sqrt(rcols[:m, g:g + 1], sq_ps[:m, g:g + 1])
            nc.vector.tensor_scalar_add(rcols[:m, g:g + 1], rcols[:m, g:g + 1], EPS)
            nc.vector.reciprocal(rcols[:m, g:g + 1], rcols[:m, g:g + 1])

        # M3 + outputs
        p_ps = ps_p.tile([128, len(mg) * NCLS], F32, tag="p")
        for g, (o, m) in enumerate(mg):
            for kk in range(4):
                nc.tensor.matmul(p_ps[:m, g * NCLS:(g + 1) * NCLS],
                                 h_sb[kk][:, o:o + m], pt_sb[kk][:],
                                 start=(kk == 0), stop=(kk == 3))
            oc = outp.tile([128, D], F32, tag="ocls")
            nc.vector.tensor_scalar(oc[:m, 0:NCLS], p_ps[:m, g * NCLS:(g + 1) * NCLS],
                                    alpha_col[:m, :], None, mult)
            nc.vector.tensor_copy(oc[:m, NCLS:D], unk_col[:m, :])
            oring_eng[g % len(oring_eng)].dma_start(
                out=ocls[col0 + o:col0 + o + m, :], in_=oc[:m, :])

            os_ = outp.tile([128, D], F32, tag="ocos")
            nc.vector.tensor_scalar(os_[:m, 0:NCLS], p_ps[:m, g * NCLS:(g + 1) * NCLS],
                                    rcols[:m, g:g + 1], None, mult)
            nc.vector.tensor_copy(os_[:m, NCLS:D], unk_col[:m, :])
            oring_eng[(g + 1) % len(oring_eng)].dma_start(
                out=ocos[col0 + o:col0 + o + m, :], in_=os_[:m, :])
        s0 += ns


def host_prep(feature, A, protos, W, b, ALPHA, UNK_SCR, np_dt=np.float16):
    """Build the 8 per-core input maps (host-side layout prep)."""
    f3 = np.ascontiguousarray(feature.reshape(NB, NC, HW).transpose(0, 2, 1)).astype(np_dt)
    a3r = A.reshape(NB, NT, HW)
    a3 = np.ascontiguousarray(a3r.transpose(0, 2, 1)).astype(np_dt)
    s = (1.0 / a3r.sum(axis=2, dtype=np.float64)).astype(np.float32)  # [NB, NT]
    wt = np.ascontiguousarray(W.T).astype(np_dt)
    pt = np.ascontiguousarray(protos.T).astype(np_dt)
    bcol = np.ascontiguousarray(b.reshape(4, 128).T).astype(np.float32)
    au = np.array([[float(ALPHA[0, 0]), float(UNK_SCR[0, 0])]], np.float32)
    in_maps = []
    for c in range(N_CORES):
        sl = slice(c * NB_C, (c + 1) * NB_C)
        in_maps.append(dict(
            ft=f3[sl].reshape(NB_C * HW, NC),
            at=a3[sl].reshape(NB_C * HW, NT),
            wt=wt, pt=pt, bcol=bcol,
            srow=s[sl].reshape(1, NB_C * NT),
            au=au,
        ))
    return in_maps


def host_pack(dense_cls, dense_cos, textLength):
    """Ragged per-sample packing (matches reference.pack)."""
    usedLen = np.minimum(textLength.astype(np.int64), NT)
    offsets = np.cumsum(usedLen) - usedLen
    b_idx, t_idx = np.nonzero(t_mask := (np.arange(NT)[None, :] < usedLen[:, None]))
    out_cls = np.zeros((NB * NT, D), np.float32)
    out_cos = np.zeros((NB * NT, D), np.float32)
    dest = offsets[b_idx] + t_idx
    src = b_idx * NT + t_idx
    out_cls[dest] = dense_cls[src]
    out_cos[dest] = dense_cos[src]
    return out_cls, out_cos


_NC_CACHE = {}


def _get_nc(dt2=mybir.dt.float16, reps=1, **kw):
    key = (str(dt2), reps, tuple(sorted(kw.items())))
    if key not in _NC_CACHE:
        _NC_CACHE[key] = build_kernel(dt2, reps, **kw)
    return _NC_CACHE[key]


FINAL_CFG = dict(dual_ring=True, out_rings="sa")


def kernel(feature, A, protos, W, b, ALPHA, UNK_SCR, textLength):
    feature = np.asarray(feature, np.float32)
    A = np.asarray(A, np.float32)
    in_maps = host_prep(np.asarray(feature, np.float32), np.asarray(A, np.float32),
                        np.asarray(protos, np.float32), np.asarray(W, np.float32),
                        np.asarray(b, np.float32), np.asarray(ALPHA, np.float32),
                        np.asarray(UNK_SCR, np.float32))
    nc = _get_nc(**FINAL_CFG)
    res = None
    for attempt in range(3):
        try:
            res = run_bass_kernel_spmd(nc, in_maps, core_ids=list(range(N_CORES)))
            break
        except Exception:  # noqa: BLE001 - transient device/tunnel hiccups
            if attempt == 2:
                raise
            import time as _time
            _time.sleep(30)
    dense_cls = np.concatenate([res.results[c]["ocls"] for c in range(N_CORES)], axis=0)
    dense_cos = np.concatenate([res.results[c]["ocos"] for c in range(N_CORES)], axis=0)
    return host_pack(dense_cls, dense_cos, np.asarray(textLength))



# revision 2
# speedup vs baseline: 1.4729x; 1.4729x over previous
"""Trainium2 Bass kernel for nn_DecoupledTextDecoder.

Reference computation (per batch sample b, nB=256, nC=512, nH*nW=512, nT=40,
nCls=97):
  A_n   = A / sum_hw(A)                       (attention normalize)
  C     = einsum('chw,thw->tc', feature_b, A_n_b)       [40, 512]
  hidden= C @ W.T + b                                   [40, 512]
  cfP   = hidden @ protos.T                             [40, 97]
  cfCos = cfP / (||hidden||_row + EPS)
  outCls= concat([cfP * ALPHA, UNK], -1); outCos = concat([cfCos, UNK], -1)
  ragged-pack the first textLength[b] rows of each sample into one buffer.

Strategy: data-parallel over nB across 8 NeuronCores (32 samples/core).
Bulk inputs (feature, A) are shipped in fp8e4 (the dominant HBM traffic —
feature alone is 8 MiB/core in fp8) in a partition-major host layout so
every DMA is a plain 2D copy with multi-KB contiguous per-partition lines.
The attention normalization is algebraically folded into a per-column scale
s[t]=1/rowsum(A) applied to C after the matmul (host computes s exactly in
fp64; normalized A itself would underflow fp8).  Per-sample matmul chain:
  M1: C^T[c,t]      = ft-chunks(lhsT, fp8) x at-chunks(rhs, fp8), acc over hw
  M2: hidden^T[c',t]= WT-chunks(lhsT) x C^T(rhs) in fp16,         acc over c
  M3: cfP[t,cls]    = hidden^T-slices(lhsT) x protos^T(rhs),      acc over c'
  hnorm^2 via DVE squares + ones-matmul partition reduce -> [t,1] layout,
  so the cfCos division and ALPHA scale are per-partition tensor_scalars.
Both outputs are packed into one [rows, 2*(nCls+1)] f32 DRAM tensor (784 B
rows, above the 512 B DMA line-rate floor; half the output DMA count).
The ragged pack is pure data movement with runtime row offsets; it is done
on the host with a vectorized scatter (the per-sample rows shard cleanly).

fp8 on feature/A costs ~1.2e-3 resid-var end-to-end (vs 2e-2 gate); the
fp16 M2/M3 stages keep everything else at ~1e-7.
"""

import numpy as np
import ml_dtypes

import concourse.bass as bass
import concourse.bacc as bacc
import concourse.tile as tile
import concourse.mybir as mybir
from concourse.bass_utils import run_bass_kernel_spmd

F32 = mybir.dt.float32
EPS = 0.0009

N_CORES = 8
NB = 256
NB_C = NB // N_CORES       # samples per core
NC = 512                   # channels
HW = 512                   # nH*nW
NT = 40                    # text steps
NCLS = 97
D = NCLS + 1
D2 = 2 * D

BLOCKS = [12, 12, 8]       # samples per block (sum = NB_C)


def _mgroups(ns):
    """Partition-dim groups of t-columns for M3 (<=128 each, 40-aligned)."""
    w = ns * NT
    out, o = [], 0
    while o < w:
        m = min(120, w - o)
        out.append((o, m))
        o += m
    return out


def build_kernel(dt2=mybir.dt.float16, reps=1, group=4, dual_ring=True,
                 ft_bufs=3, timing_mode=False, hw_loop=0,
                 out_rings="sa", dt_ft=mybir.dt.float8e4):
    """Build + compile the per-core Bass program. Returns nc.

    timing_mode=True replaces the bulk inputs with Internal DRAM scratch so
    repeated-execution benchmarks don't pay host->device re-transfers; the
    on-device HBM traffic is identical.
    """
    nc = bacc.Bacc("TRN2", target_bir_lowering=False, debug=False,
                   enable_asserts=True, num_devices=N_CORES)

    kind_b = "Internal" if timing_mode else "ExternalInput"
    # partition-major: ft[p, ((s*4+k)*NC + c)] = feature[s, c, k*128+p]
    ft = nc.dram_tensor("ft", [128, NB_C * 4 * NC], dt_ft, kind=kind_b).ap()
    # partition-major: at[p, ((s*4+k)*NT + t)] = A[s, t, k*128+p]
    at = nc.dram_tensor("at", [128, NB_C * 4 * NT], dt_ft, kind=kind_b).ap()
    wt = nc.dram_tensor("wt", [NC, NC], dt2, kind=kind_b).ap()
    pt = nc.dram_tensor("pt", [NC, NCLS], dt2, kind=kind_b).ap()
    bcol = nc.dram_tensor("bcol", [128, NC // 128], F32, kind=kind_b).ap()
    srow = nc.dram_tensor("srow", [1, NB_C * NT], F32, kind=kind_b).ap()
    au = nc.dram_tensor("au", [1, 2], F32, kind="ExternalInput").ap()
    # both outputs side by side: row = [outCls (98) | outCos (98)]
    ocat = nc.dram_tensor("ocat", [NB_C * NT, D2], F32, kind="ExternalOutput").ap()

    with tile.TileContext(nc) as tc:
        with (
            tc.tile_pool(name="consts", bufs=1) as consts,
            tc.tile_pool(name="ftp", bufs=ft_bufs) as ftp,
            tc.tile_pool(name="work", bufs=8) as work,
            tc.tile_pool(name="sqp", bufs=5) as sqp,
            tc.tile_pool(name="outp", bufs=12) as outp,
            tc.tile_pool(name="ps_ct", bufs=4, space="PSUM") as ps_ct,
            tc.tile_pool(name="ps_h", bufs=2, space="PSUM") as ps_h,
            tc.tile_pool(name="ps_p", bufs=1, space="PSUM") as ps_p,
            tc.tile_pool(name="ps_sq", bufs=1, space="PSUM") as ps_sq,
        ):
            emap = {"s": nc.sync, "a": nc.scalar, "g": nc.gpsimd}
            ring_eng = [nc.sync, nc.scalar] if dual_ring else [nc.sync]
            oring_eng = [emap[ch] for ch in out_rings]

            def emit():
                _emit_once(nc, tc, consts, ftp, work, sqp, outp,
                           ps_ct, ps_h, ps_p, ps_sq,
                           ft, at, wt, pt, bcol, srow, au, ocat, dt2,
                           group, ring_eng, oring_eng, dt_ft)

            if hw_loop:
                with tc.For_i(0, hw_loop, 1):
                    emit()
            else:
                for _ in range(reps):
                    emit()
    nc.compile()
    return nc


def _emit_once(nc, tc, consts, ftp, work, sqp, outp,
               ps_ct, ps_h, ps_p, ps_sq,
               ft, at, wt, pt, bcol, srow, au, ocat, dt2,
               group, ring_eng, oring_eng, dt_ft):
    mult = mybir.AluOpType.mult
    add = mybir.AluOpType.add

    # ---- constants -------------------------------------------------------
    ones_row = consts.tile([1, 128], F32, tag="ones_row")
    nc.vector.memset(ones_row[:], 1.0)
    ones_col = consts.tile([128, 1], F32, tag="ones_col")
    nc.vector.memset(ones_col[:], 1.0)

    # whole attention tensor up front: one 640 KB DMA, 5 KB/partition lines
    at_all = consts.tile([128, NB_C * 4 * NT], dt_ft, tag="at_all")
    ring_eng[-1].dma_start(out=at_all[:], in_=at[:, :])

    wt_sb = []
    for k in range(4):
        t = consts.tile([128, NC], dt2, tag=f"wt{k}")
        ring_eng[-1].dma_start(out=t[:], in_=wt[k * 128:(k + 1) * 128, :])
        wt_sb.append(t)
    pt_sb = []
    for k in range(4):
        t = consts.tile([128, NCLS], dt2, tag=f"pt{k}")
        ring_eng[-1].dma_start(out=t[:], in_=pt[k * 128:(k + 1) * 128, :])
        pt_sb.append(t)
    b_sb = consts.tile([128, 4], F32, tag="b")
    ring_eng[-1].dma_start(out=b_sb[:], in_=bcol[:])
    s_sb = consts.tile([1, NB_C * NT], F32, tag="s")
    ring_eng[-1].dma_start(out=s_sb[:], in_=srow[:])
    au_sb = consts.tile([1, 2], F32, tag="au")
    ring_eng[-1].dma_start(out=au_sb[:], in_=au[:])

    # Broadcast s over partitions via k=1 matmul: S_all[p, col] = s[col].
    s_all = consts.tile([128, NB_C * NT], F32, tag="s_all")
    o = 0
    while o < NB_C * NT:
        w = min(512, NB_C * NT - o)
        ps = ps_h.tile([128, 512], F32, tag="h")
        nc.tensor.matmul(ps[:, :w], ones_row[:], s_sb[:, o:o + w],
                         start=True, stop=True)
        nc.vector.tensor_copy(s_all[:, o:o + w], ps[:, :w])
        o += w
    # alpha / unk broadcast columns
    au_ps = ps_h.tile([128, 512], F32, tag="h")
    nc.tensor.matmul(au_ps[:, :2], ones_row[:], au_sb[:], start=True, stop=True)
    au_col = consts.tile([128, 2], F32, tag="au_col")
    nc.vector.tensor_copy(au_col[:], au_ps[:, :2])
    alpha_col = au_col[:, 0:1]
    unk_col = au_col[:, 1:2]

    # ---- main loop over sample blocks -----------------------------------
    s0 = 0
    gidx = 0
    gsz = group * 4 * NC
    for ns in BLOCKS:
        w = ns * NT
        col0 = s0 * NT

        # feature loads, `group` samples per DMA (plain 2D, 2KB*group lines)
        ftg = []
        for j in range(ns // group):
            base = (s0 + j * group) * 4 * NC
            ftile = ftp.tile([128, gsz], dt_ft, tag="ft")
            ring_eng[gidx % len(ring_eng)].dma_start(
                out=ftile[:], in_=ft[:, base:base + gsz])
            gidx += 1
            ftg.append(ftile)

        # M1: C_raw^T accumulated into 4 psum banks, one 40-col slice/sample
        ct_ps = [ps_ct.tile([128, 480], F32, tag="ct", name=f"ct_ps{jj}")
                 for jj in range(4)]
        for sl in range(ns):
            ftile = ftg[sl // group]
            h = sl % group
            abase = (s0 + sl) * 4 * NT
            for jj in range(4):
                for kk in range(4):
                    fo = (h * 4 + kk) * NC + jj * 128
                    nc.tensor.matmul(
                        ct_ps[jj][:, sl * NT:(sl + 1) * NT],
                        ftile[:, fo:fo + 128],
                        at_all[:, abase + kk * NT:abase + (kk + 1) * NT],
                        start=(kk == 0), stop=(kk == 3))

        # scale by s (normalization fold) + cast to dt2
        ct_sb = []
        for jj in range(4):
            t = work.tile([128, 480], dt2, tag="ctsb")
            nc.vector.tensor_tensor(t[:, :w], ct_ps[jj][:, :w],
                                    s_all[:, col0:col0 + w], mult)
            ct_sb.append(t)

        # M2: hidden^T (no bias yet), 4 psum banks
        h_sb = []
        sq = []
        for jj in range(4):
            hp = ps_h.tile([128, 480], F32, tag="h")
            for kk in range(4):
                nc.tensor.matmul(hp[:, :w], wt_sb[kk][:, jj * 128:(jj + 1) * 128],
                                 ct_sb[kk][:, :w], start=(kk == 0), stop=(kk == 3))
            # bias add on ACT during psum->sbuf copy (cast to dt2)
            hs = work.tile([128, 480], dt2, tag="hsb")
            nc.scalar.activation(hs[:, :w], hp[:, :w],
                                 mybir.ActivationFunctionType.Identity,
                                 bias=b_sb[:, jj:jj + 1])
            h_sb.append(hs)
            # squared hidden for the row norms
            st = sqp.tile([128, 480], F32, tag="sq")
            nc.vector.tensor_tensor(st[:, :w], hs[:, :w], hs[:, :w], mult)
            sq.append(st)
        nc.vector.tensor_tensor(sq[0][:, :w], sq[0][:, :w], sq[1][:, :w], add)
        nc.vector.tensor_tensor(sq[2][:, :w], sq[2][:, :w], sq[3][:, :w], add)
        nc.vector.tensor_tensor(sq[0][:, :w], sq[0][:, :w], sq[2][:, :w], add)

        mg = _mgroups(ns)
        # partition-reduce -> hnorm^2 in [t, 1] layout
        sq_ps = ps_sq.tile([128, len(mg)], F32, tag="sqc")
        for g, (o, m) in enumerate(mg):
            nc.tensor.matmul(sq_ps[:m, g:g + 1], sq[0][:, o:o + m], ones_col[:],
                             start=True, stop=True)
        # r = 1 / (sqrt(hnorm^2) + EPS)
        rcols = work.tile([128, len(mg)], F32, tag="rc")
        for g, (o, m) in enumerate(mg):
            nc.scalar.sqrt(rcols[:m, g:g + 1], sq_ps[:m, g:g + 1])
            nc.vector.tensor_scalar_add(rcols[:m, g:g + 1], rcols[:m, g:g + 1], EPS)
            nc.vector.reciprocal(rcols[:m, g:g + 1], rcols[:m, g:g + 1])

        # M3 + outputs (both outputs side by side in one tile / one DMA)
        p_ps = ps_p.tile([128, len(mg) * NCLS], F32, tag="p")
        for g, (o, m) in enumerate(mg):
            for kk in range(4):
                nc.tensor.matmul(p_ps[:m, g * NCLS:(g + 1) * NCLS],
                                 h_sb[kk][:, o:o + m], pt_sb[kk][:],
                                 start=(kk == 0), stop=(kk == 3))
            oc = outp.tile([128, D2], F32, tag="ocat")
            nc.vector.tensor_scalar(oc[:m, 0:NCLS], p_ps[:m, g * NCLS:(g + 1) * NCLS],
                                    alpha_col[:m, :], None, mult)
            nc.vector.tensor_copy(oc[:m, NCLS:D], unk_col[:m, :])
            nc.vector.tensor_scalar(oc[:m, D:D + NCLS], p_ps[:m, g * NCLS:(g + 1) * NCLS],
                                    rcols[:m, g:g + 1], None, mult)
            nc.vector.tensor_copy(oc[:m, D + NCLS:D2], unk_col[:m, :])
            oring_eng[g % len(oring_eng)].dma_start(
                out=ocat[col0 + o:col0 + o + m, :], in_=oc[:m, :])
        s0 += ns


def host_prep(feature, A, protos, W, b, ALPHA, UNK_SCR,
              np_dt=np.float16, np_dt_ft=ml_dtypes.float8_e4m3):
    """Build the 8 per-core input maps (host-side layout prep)."""
    # partition-major fp8: ftpm[p, b, k, c] = feature[b, c, k*128+p]
    f4 = feature.reshape(NB, NC, 4, 128)
    ftpm = np.ascontiguousarray(f4.transpose(3, 0, 2, 1)).astype(np_dt_ft)
    a3r = A.reshape(NB, NT, HW)
    a4 = a3r.reshape(NB, NT, 4, 128)
    atpm = np.ascontiguousarray(a4.transpose(3, 0, 2, 1)).astype(np_dt_ft)
    s = (1.0 / a3r.sum(axis=2, dtype=np.float64)).astype(np.float32)  # [NB, NT]
    wt = np.ascontiguousarray(W.T).astype(np_dt)
    pt = np.ascontiguousarray(protos.T).astype(np_dt)
    bcol = np.ascontiguousarray(b.reshape(4, 128).T).astype(np.float32)
    au = np.array([[float(ALPHA[0, 0]), float(UNK_SCR[0, 0])]], np.float32)
    in_maps = []
    for c in range(N_CORES):
        sl = slice(c * NB_C, (c + 1) * NB_C)
        in_maps.append(dict(
            ft=np.ascontiguousarray(ftpm[:, sl]).reshape(128, NB_C * 4 * NC),
            at=np.ascontiguousarray(atpm[:, sl]).reshape(128, NB_C * 4 * NT),
            wt=wt, pt=pt, bcol=bcol,
            srow=s[sl].reshape(1, NB_C * NT),
            au=au,
        ))
    return in_maps


def host_pack(dense_cls, dense_cos, textLength):
    """Ragged per-sample packing (matches reference.pack)."""
    usedLen = np.minimum(textLength.astype(np.int64), NT)
    offsets = np.cumsum(usedLen) - usedLen
    b_idx, t_idx = np.nonzero(np.arange(NT)[None, :] < usedLen[:, None])
    out_cls = np.zeros((NB * NT, D), np.float32)
    out_cos = np.zeros((NB * NT, D), np.float32)
    dest = offsets[b_idx] + t_idx
    src = b_idx * NT + t_idx
    out_cls[dest] = dense_cls[src]
    out_cos[dest] = dense_cos[src]
    return out_cls, out_cos


_NC_CACHE = {}


def _get_nc(dt2=mybir.dt.float16, reps=1, **kw):
    key = (str(dt2), reps, tuple(sorted(kw.items())))
    if key not in _NC_CACHE:
        _NC_CACHE[key] = build_kernel(dt2, reps, **kw)
    return _NC_CACHE[key]


FINAL_CFG = dict(dual_ring=True, out_rings="sa")


def kernel(feature, A, protos, W, b, ALPHA, UNK_SCR, textLength):
    in_maps = host_prep(np.asarray(feature, np.float32), np.asarray(A, np.float32),
                        np.asarray(protos, np.float32), np.asarray(W, np.float32),
                        np.asarray(b, np.float32), np.asarray(ALPHA, np.float32),
                        np.asarray(UNK_SCR, np.float32))
    nc = _get_nc(**FINAL_CFG)
    res = None
    for attempt in range(3):
        try:
            res = run_bass_kernel_spmd(nc, in_maps, core_ids=list(range(N_CORES)))
            break
        except Exception:  # noqa: BLE001 - transient device/tunnel hiccups
            if attempt == 2:
                raise
            import time as _time
            _time.sleep(30)
    dense = np.concatenate([res.results[c]["ocat"] for c in range(N_CORES)], axis=0)
    return host_pack(dense[:, :D], dense[:, D:], np.asarray(textLength))


# revision 30
# speedup vs baseline: 1.7965x; 1.2197x over previous
"""Trainium2 Bass kernel for nn_DecoupledTextDecoder.

Reference computation (per batch sample b, nB=256, nC=512, nH*nW=512, nT=40,
nCls=97):
  A_n   = A / sum_hw(A)                       (attention normalize)
  C     = einsum('chw,thw->tc', feature_b, A_n_b)       [40, 512]
  hidden= C @ W.T + b                                   [40, 512]
  cfP   = hidden @ protos.T                             [40, 97]
  cfCos = cfP / (||hidden||_row + EPS)
  outCls= concat([cfP * ALPHA, UNK], -1); outCos = concat([cfCos, UNK], -1)
  ragged-pack the first textLength[b] rows of each sample into one buffer.

Strategy: data-parallel over nB across 8 NeuronCores (32 samples/core).
Bulk inputs (feature, A) ship in fp8e4 (feature alone is 8 MiB/core) in a
partition-major host layout so every DMA is a plain 2D copy with multi-KB
contiguous per-partition lines.  All feature DMAs are issued up-front on
one HWDGE ring in need order (SBUF holds the whole 8 MiB core slice), so
the SDMA engines stream at full rate, decoupled from compute; small
constants and the per-block attention tiles ride the second HWDGE ring.
A is normalized ON THE HOST (x64 so fp8 doesn't underflow; the 1/64 is
folded into W on the host too), which deletes every on-chip scale op.
Per-sample matmul chain:
  M1: C^T[c,t]      = ft-chunks(lhsT, fp8) x at-chunks(rhs, fp8), acc over hw
       (jj-outer: each PSUM bank closes early; its psum->sbuf cast overlaps
        the next pass)
  M2: hidden^T[c',t]= WT-chunks(lhsT) x C^T(rhs) in fp16,         acc over c
  M3: cfP[t,cls]    = hidden^T-slices(lhsT) x protos^T(rhs),      acc over c'
  hidden^2 in fp16 (2x DVE), ones-matmul partition reduce -> hnorm^2 in
  [t,1] layout, so the cfCos division and ALPHA scale are per-partition
  tensor_scalars.  PSUM->SBUF casts alternate between ACT and DVE.
Software pipelining: PE program order is [M1_0, M2_0, M1_1, sqred_0, M3_0,
M2_1, M1_2, ...] so block b's cross-engine norm chain runs while the PE
streams block b+1's M1, and the final (tiny, 2-sample) block minimizes the
post-DMA tail.  Both outputs are packed into one [rows, 2*(nCls+1)] f32
DRAM tensor (784 B rows, above the 512 B DMA line-rate floor).
The ragged pack is pure data movement with runtime row offsets; it is done
on the host with a vectorized scatter (the per-sample rows shard cleanly).

fp8 on feature/A costs ~1.3e-3 resid-var end-to-end (vs 2e-2 gate); the
fp16 M2/M3 stages keep everything else at ~1e-7.
"""

import numpy as np
import ml_dtypes

import concourse.bass as bass
import concourse.bacc as bacc
import concourse.tile as tile
import concourse.mybir as mybir
from concourse.bass_utils import run_bass_kernel_spmd

F32 = mybir.dt.float32
EPS = 0.0009
ANORM = 64.0               # host A-normalization scale (folded into W)

N_CORES = 8
NB = 256
NB_C = NB // N_CORES       # samples per core
NC = 512                   # channels
HW = 512                   # nH*nW
NT = 40                    # text steps
NCLS = 97
D = NCLS + 1
D2 = 2 * D


def _mgroups(ns):
    """Partition-dim groups of t-columns for M3 (128-wide for FWL)."""
    w = ns * NT
    out, o = [], 0
    while o < w:
        m = min(128, w - o)
        out.append((o, m))
        o += m
    return out


def build_kernel(dt2=mybir.dt.float16, reps=1, dual_ring=True,
                 ft_bufs=11, timing_mode=False, hw_loop=0,
                 out_rings="s", dt_ft=mybir.dt.float8e4,
                 blocks=(12, 12, 6, 2),
                 ft_groups=(2, 2, 4, 4, 4, 4, 4, 4, 2, 2),
                 skip_load=False, skip_compute=False, stages=3):
    """Build + compile the per-core Bass program. Returns nc.

    timing_mode=True replaces the bulk inputs with Internal DRAM scratch so
    repeated-execution benchmarks don't pay host->device re-transfers; the
    on-device HBM traffic is identical.
    """
    nc = bacc.Bacc("TRN2", target_bir_lowering=False, debug=False,
                   enable_asserts=True, num_devices=N_CORES)

    kind_b = "Internal" if timing_mode else "ExternalInput"
    # partition-major: ft[p, ((s*4+k)*NC + c)] = feature[s, c, k*128+p]
    ft = nc.dram_tensor("ft", [128, NB_C * 4 * NC], dt_ft, kind=kind_b).ap()
    # partition-major, host-normalized: at[p, ((s*4+k)*NT + t)]
    #   = (A/sum_hw(A)*ANORM)[s, t, k*128+p]
    at = nc.dram_tensor("at", [128, NB_C * 4 * NT], dt_ft, kind=kind_b).ap()
    wt = nc.dram_tensor("wt", [NC, NC], dt2, kind=kind_b).ap()      # W.T/ANORM
    pt = nc.dram_tensor("pt", [NC, NCLS], dt2, kind=kind_b).ap()
    bcol = nc.dram_tensor("bcol", [128, NC // 128], F32, kind=kind_b).ap()
    au = nc.dram_tensor("au", [1, 2], F32, kind="ExternalInput").ap()
    # both outputs side by side: row = [outCls (98) | outCos (98)]
    ocat = nc.dram_tensor("ocat", [NB_C * NT, D2], F32, kind="ExternalOutput").ap()

    with tile.TileContext(nc) as tc:
        with (
            tc.tile_pool(name="consts", bufs=1) as consts,
            tc.tile_pool(name="ftp", bufs=ft_bufs) as ftp,
            tc.tile_pool(name="work", bufs=20) as work,
            tc.tile_pool(name="sqp", bufs=9) as sqp,
            tc.tile_pool(name="outp", bufs=12) as outp,
            tc.tile_pool(name="ps_ct", bufs=4, space="PSUM") as ps_ct,
            tc.tile_pool(name="ps_h", bufs=3, space="PSUM") as ps_h,
            tc.tile_pool(name="ps_p", bufs=1, space="PSUM") as ps_p,
        ):
            emap = {"s": nc.sync, "a": nc.scalar, "g": nc.gpsimd}
            ring_eng = [nc.sync, nc.scalar] if dual_ring else [nc.sync]
            oring_eng = [emap[ch] for ch in out_rings]
            fg = list(ft_groups)
            assert sum(fg) == NB_C

            def emit():
                _emit_once(nc, tc, consts, ftp, work, sqp, outp,
                           ps_ct, ps_h, ps_p,
                           ft, at, wt, pt, bcol, au, ocat, dt2,
                           fg, ring_eng, oring_eng, dt_ft,
                           list(blocks), skip_load, skip_compute, stages)

            if hw_loop:
                with tc.For_i(0, hw_loop, 1):
                    emit()
            else:
                for _ in range(reps):
                    emit()
    nc.compile()
    return nc


def _emit_once(nc, tc, consts, ftp, work, sqp, outp,
               ps_ct, ps_h, ps_p,
               ft, at, wt, pt, bcol, au, ocat, dt2,
               fg, ring_eng, oring_eng, dt_ft, blocks,
               skip_load=False, skip_compute=False, stages=3):
    mult = mybir.AluOpType.mult
    add = mybir.AluOpType.add

    class Blk:
        pass

    bs = []
    s0 = 0
    for ns in blocks:
        B = Blk()
        B.s0, B.ns, B.w, B.col0 = s0, ns, ns * NT, s0 * NT
        B.mg = _mgroups(ns)
        bs.append(B)
        s0 += ns

    # ---- bulk loads first ------------------------------------------------
    # ring 0 (sync): the feature DMAs, strictly in need-order — one HWDGE
    # queue spreads each DMA over all 16 SDMA engines, so a single ring
    # still streams at full rate and completion order matches need order.
    # First groups are small so M1 can start early.
    ftg = {}           # sample index -> (tile, sample offset within tile)
    o = 0
    for g in fg:
        gsz = g * 4 * NC
        ftile = ftp.tile([128, gsz], dt_ft, tag=f"ft{g}")
        if not skip_load:
            ring_eng[0].dma_start(
                out=ftile[:], in_=ft[:, o * 4 * NC:o * 4 * NC + gsz])
        for h in range(g):
            ftg[o + h] = (ftile, h)
        o += g

    # ring 1 (scalar): per-block attention tiles + small consts, in need
    # order.  The ACT engine has no compute duty until M2_0, so these
    # triggers never block compute.
    reng = ring_eng[-1]
    for i, B in enumerate(bs):
        B.at = consts.tile([128, B.ns * 4 * NT], dt_ft, tag=f"at{i}")
        if not skip_load:
            reng.dma_start(out=B.at[:],
                           in_=at[:, B.s0 * 4 * NT:(B.s0 + B.ns) * 4 * NT])
        if i == 0:
            au_sb = consts.tile([1, 2], F32, tag="au")
            reng.dma_start(out=au_sb[:], in_=au[:])
            b_sb = consts.tile([128, 4], F32, tag="b")
            reng.dma_start(out=b_sb[:], in_=bcol[:])
            pt_sb = []
            for k in range(4):
                t = consts.tile([128, NCLS], dt2, tag=f"pt{k}")
                reng.dma_start(out=t[:], in_=pt[k * 128:(k + 1) * 128, :])
                pt_sb.append(t)
        if i == 1:
            wt_sb = []
            for k in range(4):
                t = consts.tile([128, NC], dt2, tag=f"wt{k}")
                reng.dma_start(out=t[:], in_=wt[k * 128:(k + 1) * 128, :])
                wt_sb.append(t)

    ones_row = consts.tile([1, 128], F32, tag="ones_row")
    nc.vector.memset(ones_row[:], 1.0)
    ones_col = consts.tile([128, 1], dt2, tag="ones_col")
    nc.vector.memset(ones_col[:], 1.0)

    # alpha / unk broadcast columns (k=1 matmul)
    au_ps = ps_h.tile([128, 512], F32, tag="h")
    nc.tensor.matmul(au_ps[:, :2], ones_row[:], au_sb[:], start=True, stop=True)
    au_col = consts.tile([128, 2], F32, tag="au_col")
    nc.vector.tensor_copy(au_col[:], au_ps[:, :2])
    alpha_col = au_col[:, 0:1]
    unk_col = au_col[:, 1:2]

    if skip_compute:
        return

    def emit_m1(B):
        # M1: C^T accumulated into 4 psum banks, one 40-col slice/sample.
        # jj-outer: bank jj is fully written after pass jj, so its cast
        # overlaps the next pass instead of gating M2 at the end.
        ct_ps = [ps_ct.tile([128, 480], F32, tag="ct", name=f"ct_ps{jj}")
                 for jj in range(4)]
        B.ct_sb = []
        for jj in range(4):
            for sl in range(B.ns):
                ftile, h = ftg[B.s0 + sl]
                abase = sl * 4 * NT
                for kk in range(4):
                    fo = (h * 4 + kk) * NC + jj * 128
                    nc.tensor.matmul(
                        ct_ps[jj][:, sl * NT:(sl + 1) * NT],
                        ftile[:, fo:fo + 128],
                        B.at[:, abase + kk * NT:abase + (kk + 1) * NT],
                        start=(kk == 0), stop=(kk == 3))
            # psum -> sbuf cast to dt2, alternating ACT / DVE
            t = work.tile([128, 480], dt2, tag="ctsb")
            if jj % 2 == 0:
                nc.scalar.activation(t[:, :B.w], ct_ps[jj][:, :B.w],
                                     mybir.ActivationFunctionType.Identity)
            else:
                nc.vector.tensor_copy(t[:, :B.w], ct_ps[jj][:, :B.w])
            B.ct_sb.append(t)

    def emit_m2(B):
        # M2: hidden^T, 3 psum banks round-robin
        B.h_sb = []
        B.sq = []
        for jj in range(4):
            hp = ps_h.tile([128, 480], F32, tag="h")
            for kk in range(4):
                nc.tensor.matmul(hp[:, :B.w],
                                 wt_sb[kk][:, jj * 128:(jj + 1) * 128],
                                 B.ct_sb[kk][:, :B.w],
                                 start=(kk == 0), stop=(kk == 3))
            # bias add during psum->sbuf cast, alternating DVE / ACT
            hs = work.tile([128, 480], dt2, tag="hsb")
            if jj % 2 == 0:
                nc.vector.tensor_scalar(hs[:, :B.w], hp[:, :B.w],
                                        b_sb[:, jj:jj + 1], None, add)
            else:
                nc.scalar.activation(hs[:, :B.w], hp[:, :B.w],
                                     mybir.ActivationFunctionType.Identity,
                                     bias=b_sb[:, jj:jj + 1])
            B.h_sb.append(hs)
            # squared hidden for the row norms (fp16 end to end: 2x DVE)
            st = sqp.tile([128, 480], dt2, tag="sq")
            nc.vector.tensor_tensor(st[:, :B.w], hs[:, :B.w], hs[:, :B.w],
                                    mult)
            B.sq.append(st)
        w = B.w
        sq = B.sq
        nc.vector.tensor_tensor(sq[0][:, :w], sq[0][:, :w], sq[1][:, :w], add)
        nc.vector.tensor_tensor(sq[2][:, :w], sq[2][:, :w], sq[3][:, :w], add)
        nc.vector.tensor_tensor(sq[0][:, :w], sq[0][:, :w], sq[2][:, :w], add)

    def emit_sqred(B):
        mg = B.mg
        ng = len(mg)
        # one psum tile holds both the M3 output region and the hnorm^2
        # columns (fits one bank), freeing a bank for ps_h triple-buffering
        B.misc_ps = ps_p.tile([128, ng * NCLS + ng], F32, tag="p")
        sqv = B.misc_ps[:, ng * NCLS:ng * NCLS + ng]
        # partition-reduce -> hnorm^2 in [t, 1] layout
        for g, (o, m) in enumerate(mg):
            nc.tensor.matmul(sqv[:m, g:g + 1], B.sq[0][:, o:o + m],
                             ones_col[:], start=True, stop=True)
        # r = 1 / (sqrt(hnorm^2) + EPS), batched over the full-128 groups
        B.rcols = work.tile([128, ng], F32, tag="rc")
        nf = sum(1 for _, m in mg if m == 128)
        spans = []
        if nf:
            spans.append((0, nf, 128))
        if nf < ng:
            spans.append((nf, ng, mg[-1][1]))
        for g0, g1, m in spans:
            nc.scalar.sqrt(B.rcols[:m, g0:g1], sqv[:m, g0:g1])
            nc.vector.tensor_scalar_add(B.rcols[:m, g0:g1], B.rcols[:m, g0:g1],
                                        EPS)
            nc.vector.reciprocal(B.rcols[:m, g0:g1], B.rcols[:m, g0:g1])

    og = [0]

    def emit_m3(B):
        mg = B.mg
        p_ps = B.misc_ps
        # M3 + outputs (both outputs side by side in one tile / one DMA)
        for g, (o, m) in enumerate(mg):
            for kk in range(4):
                nc.tensor.matmul(p_ps[:m, g * NCLS:(g + 1) * NCLS],
                                 B.h_sb[kk][:, o:o + m], pt_sb[kk][:],
                                 start=(kk == 0), stop=(kk == 3))
            oc = outp.tile([128, D2], F32, tag="ocat")
            nc.vector.tensor_scalar(oc[:m, 0:NCLS],
                                    p_ps[:m, g * NCLS:(g + 1) * NCLS],
                                    alpha_col[:m, :], None, mult)
            nc.vector.tensor_copy(oc[:m, NCLS:D], unk_col[:m, :])
            nc.vector.tensor_scalar(oc[:m, D:D + NCLS],
                                    p_ps[:m, g * NCLS:(g + 1) * NCLS],
                                    B.rcols[:m, g:g + 1], None, mult)
            nc.vector.tensor_copy(oc[:m, D + NCLS:D2], unk_col[:m, :])
            oring_eng[og[0] % len(oring_eng)].dma_start(
                out=ocat[B.col0 + o:B.col0 + o + m, :], in_=oc[:m, :])
            og[0] += 1

    prev = None
    for B in bs:
        emit_m1(B)
        if stages >= 3 and prev is not None:
            emit_sqred(prev)
            emit_m3(prev)
        if stages >= 2:
            emit_m2(B)
        prev = B
    if stages >= 3:
        emit_sqred(prev)
        emit_m3(prev)


def host_prep(feature, A, protos, W, b, ALPHA, UNK_SCR,
              np_dt=np.float16, np_dt_ft=ml_dtypes.float8_e4m3):
    """Build the 8 per-core input maps (host-side layout prep)."""
    # partition-major fp8: ftpm[p, b, k, c] = feature[b, c, k*128+p]
    f4 = feature.reshape(NB, NC, 4, 128)
    ftpm = np.ascontiguousarray(f4.transpose(3, 0, 2, 1)).astype(np_dt_ft)
    # A normalized on host, scaled by ANORM to stay in fp8's range; the
    # 1/ANORM is folded into W below.
    a3r = A.reshape(NB, NT, HW)
    an = (a3r * (ANORM / a3r.sum(axis=2, keepdims=True, dtype=np.float64))
          ).astype(np.float32)
    a4 = an.reshape(NB, NT, 4, 128)
    atpm = np.ascontiguousarray(a4.transpose(3, 0, 2, 1)).astype(np_dt_ft)
    wt = np.ascontiguousarray(W.T / ANORM).astype(np_dt)
    pt = np.ascontiguousarray(protos.T).astype(np_dt)
    bcol = np.ascontiguousarray(b.reshape(4, 128).T).astype(np.float32)
    au = np.array([[float(ALPHA[0, 0]), float(UNK_SCR[0, 0])]], np.float32)
    in_maps = []
    for c in range(N_CORES):
        sl = slice(c * NB_C, (c + 1) * NB_C)
        in_maps.append(dict(
            ft=np.ascontiguousarray(ftpm[:, sl]).reshape(128, NB_C * 4 * NC),
            at=np.ascontiguousarray(atpm[:, sl]).reshape(128, NB_C * 4 * NT),
            wt=wt, pt=pt, bcol=bcol,
            au=au,
        ))
    return in_maps


def host_pack(dense_cls, dense_cos, textLength):
    """Ragged per-sample packing (matches reference.pack)."""
    usedLen = np.minimum(textLength.astype(np.int64), NT)
    offsets = np.cumsum(usedLen) - usedLen
    b_idx, t_idx = np.nonzero(np.arange(NT)[None, :] < usedLen[:, None])
    out_cls = np.zeros((NB * NT, D), np.float32)
    out_cos = np.zeros((NB * NT, D), np.float32)
    dest = offsets[b_idx] + t_idx
    src = b_idx * NT + t_idx
    out_cls[dest] = dense_cls[src]
    out_cos[dest] = dense_cos[src]
    return out_cls, out_cos


_NC_CACHE = {}


def _get_nc(dt2=mybir.dt.float16, reps=1, **kw):
    key = (str(dt2), reps, tuple(sorted(str(i) for i in kw.items())))
    if key not in _NC_CACHE:
        _NC_CACHE[key] = build_kernel(dt2, reps, **kw)
    return _NC_CACHE[key]


FINAL_CFG = dict(out_rings="s")


def kernel(feature, A, protos, W, b, ALPHA, UNK_SCR, textLength):
    in_maps = host_prep(np.asarray(feature, np.float32), np.asarray(A, np.float32),
                        np.asarray(protos, np.float32), np.asarray(W, np.float32),
                        np.asarray(b, np.float32), np.asarray(ALPHA, np.float32),
                        np.asarray(UNK_SCR, np.float32))
    nc = _get_nc(**FINAL_CFG)
    res = None
    for attempt in range(3):
        try:
            res = run_bass_kernel_spmd(nc, in_maps, core_ids=list(range(N_CORES)))
            break
        except Exception:  # noqa: BLE001 - transient device/tunnel hiccups
            if attempt == 2:
                raise
            import time as _time
            _time.sleep(30)
    dense = np.concatenate([res.results[c]["ocat"] for c in range(N_CORES)], axis=0)
    return host_pack(dense[:, :D], dense[:, D:], np.asarray(textLength))


# revision 39
# speedup vs baseline: 1.8093x; 1.0071x over previous
"""Trainium2 Bass kernel for nn_DecoupledTextDecoder.

Reference computation (per batch sample b, nB=256, nC=512, nH*nW=512, nT=40,
nCls=97):
  A_n   = A / sum_hw(A)                       (attention normalize)
  C     = einsum('chw,thw->tc', feature_b, A_n_b)       [40, 512]
  hidden= C @ W.T + b                                   [40, 512]
  cfP   = hidden @ protos.T                             [40, 97]
  cfCos = cfP / (||hidden||_row + EPS)
  outCls= concat([cfP * ALPHA, UNK], -1); outCos = concat([cfCos, UNK], -1)
  ragged-pack the first textLength[b] rows of each sample into one buffer.

Strategy: data-parallel over nB across 8 NeuronCores (32 samples/core).
Bulk inputs (feature, A) ship in fp8e4 (feature alone is 8 MiB/core) in a
partition-major host layout so every DMA is a plain 2D copy with multi-KB
contiguous per-partition lines.  All feature DMAs are issued up-front on
one HWDGE ring in need order (SBUF holds the whole 8 MiB core slice), so
the SDMA engines stream at full rate, decoupled from compute; small
constants and the per-block attention tiles ride the second HWDGE ring.
A is normalized ON THE HOST (x64 so fp8 doesn't underflow; the 1/64 is
folded into W on the host too), which deletes every on-chip scale op.
Per-sample matmul chain:
  M1: C^T[c,t]      = ft-chunks(lhsT, fp8) x at-chunks(rhs, fp8), acc over hw
       (jj-outer: each PSUM bank closes early; its psum->sbuf cast overlaps
        the next pass)
  M2: hidden^T[c',t]= WT-chunks(lhsT) x C^T(rhs) in fp16,         acc over c
  M3: cfP[t,cls]    = hidden^T-slices(lhsT) x protos^T(rhs),      acc over c'
  hidden^2 in fp16 (2x DVE), ones-matmul partition reduce -> hnorm^2 in
  [t,1] layout, so the cfCos division and ALPHA scale are per-partition
  tensor_scalars.  PSUM->SBUF casts alternate between ACT and DVE.
Software pipelining: PE program order is [M1_0, M2_0, M1_1, sqred_0, M3_0,
M2_1, M1_2, ...] so block b's cross-engine norm chain runs while the PE
streams block b+1's M1, and the final (tiny, 2-sample) block minimizes the
post-DMA tail.  Both outputs are packed into one [rows, 2*(nCls+1)] f32
DRAM tensor (784 B rows, above the 512 B DMA line-rate floor).
The ragged pack is pure data movement with runtime row offsets; it is done
on the host with a vectorized scatter (the per-sample rows shard cleanly).

fp8 on feature/A costs ~1.3e-3 resid-var end-to-end (vs 2e-2 gate); the
fp16 M2/M3 stages keep everything else at ~1e-7.
"""

import numpy as np
import ml_dtypes

import concourse.bass as bass
import concourse.bacc as bacc
import concourse.tile as tile
import concourse.mybir as mybir
from concourse.bass_utils import run_bass_kernel_spmd

F32 = mybir.dt.float32
EPS = 0.0009
ANORM = 64.0               # host A-normalization scale (folded into W)

N_CORES = 8
NB = 256
NB_C = NB // N_CORES       # samples per core
NC = 512                   # channels
HW = 512                   # nH*nW
NT = 40                    # text steps
NCLS = 97
D = NCLS + 1
D2 = 2 * D


def _mgroups(ns):
    """Partition-dim groups of t-columns for M3 (128-wide for FWL)."""
    w = ns * NT
    out, o = [], 0
    while o < w:
        m = min(128, w - o)
        out.append((o, m))
        o += m
    return out


def build_kernel(dt2=mybir.dt.float16, reps=1, dual_ring=True,
                 ft_bufs=11, timing_mode=False, hw_loop=0,
                 out_rings="s", dt_ft=mybir.dt.float8e4,
                 blocks=(12, 12, 6, 2),
                 ft_groups=(2, 2, 4, 4, 4, 4, 4, 4, 2, 2),
                 skip_load=False, skip_compute=False, stages=3):
    """Build + compile the per-core Bass program. Returns nc.

    timing_mode=True replaces the bulk inputs with Internal DRAM scratch so
    repeated-execution benchmarks don't pay host->device re-transfers; the
    on-device HBM traffic is identical.
    """
    nc = bacc.Bacc("TRN2", target_bir_lowering=False, debug=False,
                   enable_asserts=True, num_devices=N_CORES)

    kind_b = "Internal" if timing_mode else "ExternalInput"
    # partition-major: ft[p, ((s*4+k)*NC + c)] = feature[s, c, k*128+p]
    ft = nc.dram_tensor("ft", [128, NB_C * 4 * NC], dt_ft, kind=kind_b).ap()
    # partition-major, host-normalized: at[p, ((s*4+k)*NT + t)]
    #   = (A/sum_hw(A)*ANORM)[s, t, k*128+p]
    at = nc.dram_tensor("at", [128, NB_C * 4 * NT], dt_ft, kind=kind_b).ap()
    wt = nc.dram_tensor("wt", [NC, NC], dt2, kind=kind_b).ap()      # W.T/ANORM
    pt = nc.dram_tensor("pt", [NC, NCLS], dt2, kind=kind_b).ap()
    bcol = nc.dram_tensor("bcol", [128, NC // 128], F32, kind=kind_b).ap()
    au = nc.dram_tensor("au", [1, 2], F32, kind="ExternalInput").ap()
    # both outputs side by side: row = [outCls (98) | outCos (98)]
    ocat = nc.dram_tensor("ocat", [NB_C * NT, D2], F32, kind="ExternalOutput").ap()

    with tile.TileContext(nc) as tc:
        with (
            tc.tile_pool(name="consts", bufs=1) as consts,
            tc.tile_pool(name="ftp", bufs=ft_bufs) as ftp,
            tc.tile_pool(name="work", bufs=20) as work,
            tc.tile_pool(name="sqp", bufs=9) as sqp,
            tc.tile_pool(name="outp", bufs=12) as outp,
            tc.tile_pool(name="ps_ct", bufs=4, space="PSUM") as ps_ct,
            tc.tile_pool(name="ps_h", bufs=3, space="PSUM") as ps_h,
            tc.tile_pool(name="ps_p", bufs=1, space="PSUM") as ps_p,
        ):
            emap = {"s": nc.sync, "a": nc.scalar, "g": nc.gpsimd}
            ring_eng = [nc.sync, nc.scalar] if dual_ring else [nc.sync]
            oring_eng = [emap[ch] for ch in out_rings]
            fg = list(ft_groups)
            assert sum(fg) == NB_C

            def emit():
                _emit_once(nc, tc, consts, ftp, work, sqp, outp,
                           ps_ct, ps_h, ps_p,
                           ft, at, wt, pt, bcol, au, ocat, dt2,
                           fg, ring_eng, oring_eng, dt_ft,
                           list(blocks), skip_load, skip_compute, stages)

            if hw_loop:
                with tc.For_i(0, hw_loop, 1):
                    emit()
            else:
                for _ in range(reps):
                    emit()
    nc.compile()
    return nc


def _emit_once(nc, tc, consts, ftp, work, sqp, outp,
               ps_ct, ps_h, ps_p,
               ft, at, wt, pt, bcol, au, ocat, dt2,
               fg, ring_eng, oring_eng, dt_ft, blocks,
               skip_load=False, skip_compute=False, stages=3):
    mult = mybir.AluOpType.mult
    add = mybir.AluOpType.add

    class Blk:
        pass

    bs = []
    s0 = 0
    for ns in blocks:
        B = Blk()
        B.s0, B.ns, B.w, B.col0 = s0, ns, ns * NT, s0 * NT
        B.mg = _mgroups(ns)
        bs.append(B)
        s0 += ns

    # ---- bulk loads first ------------------------------------------------
    # ring 0 (sync): the feature DMAs, strictly in need-order — one HWDGE
    # queue spreads each DMA over all 16 SDMA engines, so a single ring
    # still streams at full rate and completion order matches need order.
    # First groups are small so M1 can start early.
    ftg = {}           # sample index -> (tile, sample offset within tile)
    o = 0
    for g in fg:
        gsz = g * 4 * NC
        ftile = ftp.tile([128, gsz], dt_ft, tag=f"ft{g}")
        if not skip_load:
            ring_eng[0].dma_start(
                out=ftile[:], in_=ft[:, o * 4 * NC:o * 4 * NC + gsz])
        for h in range(g):
            ftg[o + h] = (ftile, h)
        o += g

    # ring 1 (scalar): per-block attention tiles + small consts, in need
    # order.  The ACT engine has no compute duty until M2_0, so these
    # triggers never block compute.
    reng = ring_eng[-1]
    au_sb = consts.tile([1, 2], F32, tag="au")
    reng.dma_start(out=au_sb[:], in_=au[:])
    for i, B in enumerate(bs):
        B.at = consts.tile([128, B.ns * 4 * NT], dt_ft, tag=f"at{i}")
        if not skip_load:
            reng.dma_start(out=B.at[:],
                           in_=at[:, B.s0 * 4 * NT:(B.s0 + B.ns) * 4 * NT])
        if i == 0:
            b_sb = consts.tile([128, 4], F32, tag="b")
            reng.dma_start(out=b_sb[:], in_=bcol[:])
            pt_sb = []
            for k in range(4):
                t = consts.tile([128, NCLS], dt2, tag=f"pt{k}")
                reng.dma_start(out=t[:], in_=pt[k * 128:(k + 1) * 128, :])
                pt_sb.append(t)
        if i == 1:
            wt_sb = []
            for k in range(4):
                t = consts.tile([128, NC], dt2, tag=f"wt{k}")
                reng.dma_start(out=t[:], in_=wt[k * 128:(k + 1) * 128, :])
                wt_sb.append(t)

    ones_row = consts.tile([1, 128], F32, tag="ones_row")
    nc.vector.memset(ones_row[:], 1.0)
    ones_col = consts.tile([128, 1], dt2, tag="ones_col")
    nc.vector.memset(ones_col[:], 1.0)

    # alpha / unk broadcast columns (k=1 matmul)
    au_ps = ps_h.tile([128, 512], F32, tag="h")
    nc.tensor.matmul(au_ps[:, :2], ones_row[:], au_sb[:], start=True, stop=True)
    au_col = consts.tile([128, 2], F32, tag="au_col")
    nc.vector.tensor_copy(au_col[:], au_ps[:, :2])
    alpha_col = au_col[:, 0:1]
    unk_col = au_col[:, 1:2]

    if skip_compute:
        return

    def emit_m1(B):
        # M1: C^T accumulated into 4 psum banks, one 40-col slice/sample.
        # jj-outer: bank jj is fully written after pass jj, so its cast
        # overlaps the next pass instead of gating M2 at the end.
        ct_ps = [ps_ct.tile([128, 480], F32, tag="ct", name=f"ct_ps{jj}")
                 for jj in range(4)]
        B.ct_sb = []
        for jj in range(4):
            bank = ct_ps[jj]
            for sl in range(B.ns):
                ftile, h = ftg[B.s0 + sl]
                abase = sl * 4 * NT
                for kk in range(4):
                    fo = (h * 4 + kk) * NC + jj * 128
                    nc.tensor.matmul(
                        bank[:, sl * NT:(sl + 1) * NT],
                        ftile[:, fo:fo + 128],
                        B.at[:, abase + kk * NT:abase + (kk + 1) * NT],
                        start=(kk == 0), stop=(kk == 3))
            # psum -> sbuf cast to dt2, alternating ACT / DVE
            t = work.tile([128, 480], dt2, tag="ctsb")
            if jj % 2 == 0:
                nc.scalar.activation(t[:, :B.w], bank[:, :B.w],
                                     mybir.ActivationFunctionType.Identity)
            else:
                nc.vector.tensor_copy(t[:, :B.w], bank[:, :B.w])
            B.ct_sb.append(t)

    def emit_m2(B):
        # M2: hidden^T, psum banks round-robin, one jj-quarter at a time
        B.h_sb = []
        B.sq = []
        for jj in range(4):
            hp = ps_h.tile([128, 480], F32, tag="h")
            for kk in range(4):
                nc.tensor.matmul(hp[:, :B.w],
                                 wt_sb[kk][:, jj * 128:(jj + 1) * 128],
                                 B.ct_sb[kk][:, :B.w],
                                 start=(kk == 0), stop=(kk == 3))
            # bias add during psum->sbuf cast, alternating DVE / ACT
            hs = work.tile([128, 480], dt2, tag="hsb")
            if jj % 2 == 0:
                nc.vector.tensor_scalar(hs[:, :B.w], hp[:, :B.w],
                                        b_sb[:, jj:jj + 1], None, add)
            else:
                nc.scalar.activation(hs[:, :B.w], hp[:, :B.w],
                                     mybir.ActivationFunctionType.Identity,
                                     bias=b_sb[:, jj:jj + 1])
            B.h_sb.append(hs)
            # squared hidden for the row norms (fp16 end to end: 2x DVE)
            st = sqp.tile([128, 480], dt2, tag="sq")
            nc.vector.tensor_tensor(st[:, :B.w], hs[:, :B.w], hs[:, :B.w],
                                    mult)
            B.sq.append(st)
        w = B.w
        sq = B.sq
        nc.vector.tensor_tensor(sq[0][:, :w], sq[0][:, :w], sq[1][:, :w], add)
        nc.vector.tensor_tensor(sq[2][:, :w], sq[2][:, :w], sq[3][:, :w], add)
        nc.vector.tensor_tensor(sq[0][:, :w], sq[0][:, :w], sq[2][:, :w], add)

    def emit_sqred(B):
        mg = B.mg
        ng = len(mg)
        # one psum tile holds both the M3 output region and the hnorm^2
        # columns (fits one bank), freeing a bank for ps_h triple-buffering
        B.misc_ps = ps_p.tile([128, ng * NCLS + ng], F32, tag="p")
        sqv = B.misc_ps[:, ng * NCLS:ng * NCLS + ng]
        # partition-reduce -> hnorm^2 in [t, 1] layout
        for g, (o, m) in enumerate(mg):
            nc.tensor.matmul(sqv[:m, g:g + 1], B.sq[0][:, o:o + m],
                             ones_col[:], start=True, stop=True)
        # r = 1 / (sqrt(hnorm^2) + EPS), batched over the full-128 groups
        B.rcols = work.tile([128, ng], F32, tag="rc")
        nf = sum(1 for _, m in mg if m == 128)
        spans = []
        if nf:
            spans.append((0, nf, 128))
        if nf < ng:
            spans.append((nf, ng, mg[-1][1]))
        for g0, g1, m in spans:
            nc.scalar.sqrt(B.rcols[:m, g0:g1], sqv[:m, g0:g1])
            nc.vector.tensor_scalar_add(B.rcols[:m, g0:g1], B.rcols[:m, g0:g1],
                                        EPS)
            nc.vector.reciprocal(B.rcols[:m, g0:g1], B.rcols[:m, g0:g1])

    og = [0]

    def emit_m3_g(B, g):
        mg = B.mg
        p_ps = B.misc_ps
        o, m = mg[g]
        # M3 + outputs (both outputs side by side in one tile / one DMA)
        for kk in range(4):
            nc.tensor.matmul(p_ps[:m, g * NCLS:(g + 1) * NCLS],
                             B.h_sb[kk][:, o:o + m], pt_sb[kk][:],
                             start=(kk == 0), stop=(kk == 3))
        oc = outp.tile([128, D2], F32, tag="ocat")
        nc.vector.tensor_scalar(oc[:m, 0:NCLS],
                                p_ps[:m, g * NCLS:(g + 1) * NCLS],
                                alpha_col[:m, :], None, mult)
        nc.vector.tensor_copy(oc[:m, NCLS:D], unk_col[:m, :])
        nc.vector.tensor_scalar(oc[:m, D:D + NCLS],
                                p_ps[:m, g * NCLS:(g + 1) * NCLS],
                                B.rcols[:m, g:g + 1], None, mult)
        nc.vector.tensor_copy(oc[:m, D + NCLS:D2], unk_col[:m, :])
        oring_eng[og[0] % len(oring_eng)].dma_start(
            out=ocat[B.col0 + o:B.col0 + o + m, :], in_=oc[:m, :])
        og[0] += 1

    prev = None
    for B in bs:
        emit_m1(B)
        if stages >= 3 and prev is not None:
            emit_sqred(prev)
            for g in range(len(prev.mg)):
                emit_m3_g(prev, g)
        if stages >= 2:
            emit_m2(B)
        prev = B
    if stages >= 3:
        emit_sqred(prev)
        for g in range(len(prev.mg)):
            emit_m3_g(prev, g)


def host_prep(feature, A, protos, W, b, ALPHA, UNK_SCR,
              np_dt=np.float16, np_dt_ft=ml_dtypes.float8_e4m3):
    """Build the 8 per-core input maps (host-side layout prep)."""
    # partition-major fp8: ftpm[p, b, k, c] = feature[b, c, k*128+p]
    f4 = feature.reshape(NB, NC, 4, 128)
    ftpm = np.ascontiguousarray(f4.transpose(3, 0, 2, 1)).astype(np_dt_ft)
    # A normalized on host, scaled by ANORM to stay in fp8's range; the
    # 1/ANORM is folded into W below.
    a3r = A.reshape(NB, NT, HW)
    an = (a3r * (ANORM / a3r.sum(axis=2, keepdims=True, dtype=np.float64))
          ).astype(np.float32)
    a4 = an.reshape(NB, NT, 4, 128)
    atpm = np.ascontiguousarray(a4.transpose(3, 0, 2, 1)).astype(np_dt_ft)
    wt = np.ascontiguousarray(W.T / ANORM).astype(np_dt)
    pt = np.ascontiguousarray(protos.T).astype(np_dt)
    bcol = np.ascontiguousarray(b.reshape(4, 128).T).astype(np.float32)
    au = np.array([[float(ALPHA[0, 0]), float(UNK_SCR[0, 0])]], np.float32)
    in_maps = []
    for c in range(N_CORES):
        sl = slice(c * NB_C, (c + 1) * NB_C)
        in_maps.append(dict(
            ft=np.ascontiguousarray(ftpm[:, sl]).reshape(128, NB_C * 4 * NC),
            at=np.ascontiguousarray(atpm[:, sl]).reshape(128, NB_C * 4 * NT),
            wt=wt, pt=pt, bcol=bcol,
            au=au,
        ))
    return in_maps


def host_pack(dense_cls, dense_cos, textLength):
    """Ragged per-sample packing (matches reference.pack)."""
    usedLen = np.minimum(textLength.astype(np.int64), NT)
    offsets = np.cumsum(usedLen) - usedLen
    b_idx, t_idx = np.nonzero(np.arange(NT)[None, :] < usedLen[:, None])
    out_cls = np.zeros((NB * NT, D), np.float32)
    out_cos = np.zeros((NB * NT, D), np.float32)
    dest = offsets[b_idx] + t_idx
    src = b_idx * NT + t_idx
    out_cls[dest] = dense_cls[src]
    out_cos[dest] = dense_cos[src]
    return out_cls, out_cos


_NC_CACHE = {}


def _get_nc(dt2=mybir.dt.float16, reps=1, **kw):
    key = (str(dt2), reps, tuple(sorted(str(i) for i in kw.items())))
    if key not in _NC_CACHE:
        _NC_CACHE[key] = build_kernel(dt2, reps, **kw)
    return _NC_CACHE[key]


FINAL_CFG = dict(out_rings="s")


def kernel(feature, A, protos, W, b, ALPHA, UNK_SCR, textLength):
    in_maps = host_prep(np.asarray(feature, np.float32), np.asarray(A, np.float32),
                        np.asarray(protos, np.float32), np.asarray(W, np.float32),
                        np.asarray(b, np.float32), np.asarray(ALPHA, np.float32),
                        np.asarray(UNK_SCR, np.float32))
    nc = _get_nc(**FINAL_CFG)
    res = None
    for attempt in range(3):
        try:
            res = run_bass_kernel_spmd(nc, in_maps, core_ids=list(range(N_CORES)))
            break
        except Exception:  # noqa: BLE001 - transient device/tunnel hiccups
            if attempt == 2:
                raise
            import time as _time
            _time.sleep(30)
    dense = np.concatenate([res.results[c]["ocat"] for c in range(N_CORES)], axis=0)
    return host_pack(dense[:, :D], dense[:, D:], np.asarray(textLength))
